# revision 1
# baseline (speedup 1.0000x reference)
"""Trainium2 Bass kernel v3 for nn_LocalCrossAttention (chunked local cross-attn).

Problem (hardcoded): B=2, T=S=8192, HIDDEN=512, NUM_HEADS=8, HEAD_SIZE=64,
CHUNK=128, N_BEFORE=1, N_AFTER=0, attention_mask == ones.

Sharding: 8 cores = batch(2) x sequence-quarters(4). Each core: 2048 query
rows (16 chunks) of one batch element, all 8 heads, 1-chunk halo of encoder
rows. Quarter 0's halo is structurally fully masked -> zeros + zero mask.

Design (CoreSim-driven; ~81us/core in the cost-model sim vs ~145us for the
original staged version; HW-verified, L2 ~5.6e-3):
  - DIRECT-LAYOUT PV: the exp tile pb (keys on partitions) is the STATIONARY
    matmul operand and V the moving one, so out = pb.T @ V lands directly as
    [query, head-dim] in PSUM. No transposes, no staging copies. V carries a
    ones column, so PSUM col 64 is the softmax denominator: normalization is
    one batched reciprocal + tensor_scalar_mul per chunk.
  - per key chunk: ONE 2-bank PSUM score tile (the HW cannot drain two
    different PE row-offsets into one bank - the two heads sit at row
    offsets 0/64), ONE strided exp (ACT), ONE strided mask-mul (GPSIMD/Pool,
    which is otherwise idle and may not touch PSUM).
  - per-chunk PV accumulation groups run strictly sequentially per bank
    (start->stop per head) - concurrent pending groups in one bank are
    illegal.
  - projections stream as 26 "units" per head-pair phase, pumped just-in-time
    (need-based + linear floor + lookahead) between attention bodies so the
    PE never starves while ACT runs exp; phases pump across boundaries.
  - chunk-major contiguous SBUF/DRAM layouts for every input DMA: interval-
    hull dependency tracking otherwise fabricates WAR chains that head-of-
    line-block the in-order DMA queue.
  - input DMAs ordered to match phase-0 consumption (first matmul at ~3us);
    PE warmup matmuls during the DMA lead-in; per-chunk output DMA (bf16,
    host casts to fp32) overlapped with compute; final group drained
    per-chunk to shorten the epilogue.
"""

import os
import sys

import numpy as np
import ml_dtypes

for _p in ("/opt/trn_rl_repo",):
    if _p not in sys.path and os.path.isdir(_p):
        sys.path.append(_p)

import concourse.bass as bass
import concourse.bacc as bacc
import concourse.mybir as mybir
from concourse.tile import TileContext
from concourse.bass_utils import run_bass_kernel_spmd

from contextlib import ExitStack

BF16 = ml_dtypes.bfloat16
DT_BF = mybir.dt.bfloat16
DT_F32 = mybir.dt.float32
EXP = mybir.ActivationFunctionType.Exp

B, T, H = 2, 8192, 512
NH, DH, C = 8, 64, 128
ROWS = 2048          # query rows per core
NCH = ROWS // C      # 16 local query chunks
EROWS = ROWS + C     # encoder rows per core incl. halo
NKC = EROWS // C     # 17 key chunks
VST = DH + 2         # v stride per head (64 vals + ones col + pad for alignment)

_CACHED = {}


def _build_program():
    nc = bacc.Bacc("TRN2", target_bir_lowering=False, debug=False,
                   enable_asserts=False, num_devices=8)

    xt_dec = nc.dram_tensor("xt_dec", [C, 4 * ROWS], DT_BF, kind="ExternalInput").ap()
    xt_enc = nc.dram_tensor("xt_enc", [C, 4 * EROWS], DT_BF, kind="ExternalInput").ap()
    wqt = nc.dram_tensor("wqt", [C, 4 * H], DT_BF, kind="ExternalInput").ap()
    wkt = nc.dram_tensor("wkt", [C, 4 * H], DT_BF, kind="ExternalInput").ap()
    wvt = nc.dram_tensor("wvt", [C, 4 * H], DT_BF, kind="ExternalInput").ap()
    trm = nc.dram_tensor("trm", [C, 2 * C], DT_BF, kind="ExternalInput").ap()
    m0 = nc.dram_tensor("m0", [C, 2 * C], DT_BF, kind="ExternalInput").ap()
    iden = nc.dram_tensor("iden", [C, C], DT_BF, kind="ExternalInput").ap()
    out_d = nc.dram_tensor("out", [ROWS, H], DT_BF, kind="ExternalOutput").ap()

    with TileContext(nc) as tc, ExitStack() as ctx:
        const = ctx.enter_context(tc.tile_pool(name="const", bufs=1))

        # ---- resident SBUF tensors ----
        xtd = const.tile([C, 4 * ROWS], DT_BF, name="xtd")
        xte = const.tile([C, 4 * EROWS], DT_BF, name="xte")
        wq = const.tile([C, 4 * H], DT_BF, name="wq")
        wk = const.tile([C, 4 * H], DT_BF, name="wk")
        wv = const.tile([C, 4 * H], DT_BF, name="wv")
        trm_sb = const.tile([C, 2 * C], DT_BF, name="trm_sb")
        m0_sb = const.tile([C, 2 * C], DT_BF, name="m0_sb")
        id_sb = const.tile([C, C], DT_BF, name="id_sb")
        qt = const.tile([C, 4 * ROWS], DT_BF, name="qt")
        kt = const.tile([C, 4 * EROWS], DT_BF, name="kt")
        v4 = [const.tile([C, NKC * 2 * VST], DT_BF, name=f"v{hp}") for hp in range(4)]
        outg = [const.tile([C, 4 * H], DT_BF, name=f"og{g}") for g in range(4)]

        # ones columns of v tiles (d==64 of each head slot; 65 is pad)
        for hp in range(4):
            vv = v4[hp].rearrange("p (n o) -> p n o", o=VST)
            nc.vector.memset(vv[:, :, DH:DH + 2], 1.0)

        # ---- input DMAs ----
        # All SBUF layouts are chunk-major so each DMA writes one contiguous
        # interval: interval-hull dependency tracking then never creates
        # false WAR chains between later input DMAs and earlier reads.
        def chunk(dram, sb, o, w):
            nc.sync.dma_start(sb[:, o:o + w], dram[:, o:o + w])

        # ordered to match the just-in-time consumption of phase 0
        chunk(wqt, wq, 0, 512)
        chunk(xt_dec, xtd, 0, 2048)
        chunk(wkt, wk, 0, 512)
        chunk(xt_enc, xte, 0, 2048)
        chunk(wvt, wv, 0, 512)
        nc.sync.dma_start(trm_sb[:], trm[:])
        nc.sync.dma_start(m0_sb[:], m0[:])
        nc.sync.dma_start(id_sb[:], iden[:])
        chunk(xt_dec, xtd, 2048, 2048)
        chunk(xt_enc, xte, 2048, 2048)
        for jb in (1, 2):
            chunk(wqt, wq, 512 * jb, 512)
            chunk(wkt, wk, 512 * jb, 512)
            chunk(wvt, wv, 512 * jb, 512)
            chunk(xt_dec, xtd, 2048 * (jb + 1), 2048)
            chunk(xt_enc, xte, 2048 * (jb + 1), 2048)
        chunk(xt_enc, xte, 8192, 512)
        chunk(wqt, wq, 1536, 512)
        chunk(wkt, wk, 1536, 512)
        chunk(wvt, wv, 1536, 512)

        # ---- engine-rotating PSUM->SBUF copy helper (weighted by size) ----
        ROT_BIG = [nc.scalar, nc.vector]
        ROT_SMALL = [nc.vector]
        ROT_TR = [nc.vector, nc.scalar]

        def cp(dst, src, kind="big", rots={"big": [0], "small": [0], "tr": [0]}):
            lst = {"big": ROT_BIG, "small": ROT_SMALL, "tr": ROT_TR}[kind]
            r = rots[kind]
            e = lst[r[0] % len(lst)]
            r[0] += 1
            if e is nc.scalar:
                e.copy(dst, src)
            else:
                e.tensor_copy(dst, src)

        pj = ctx.enter_context(tc.tile_pool(name="pj", bufs=2, space="PSUM"))
        scp = ctx.enter_context(tc.tile_pool(name="scp", bufs=2, space="PSUM"))
        pvp = ctx.enter_context(tc.tile_pool(name="pvp", bufs=2, space="PSUM"))
        pbp = ctx.enter_context(tc.tile_pool(name="pbp", bufs=4))
        rcp = ctx.enter_context(tc.tile_pool(name="rcp", bufs=4))

        # ---- PE warmup: dummy matmuls while input DMAs stream ----
        warm = const.tile([C, 512], DT_BF, name="warm")
        nc.vector.memset(warm[:], 0.0)
        wps = pj.tile([C, 512], DT_F32, name="pj")
        for i in range(5):
            nc.tensor.matmul(wps[:], lhsT=warm[:, 0:C], rhs=warm[:],
                             start=(i == 0), stop=(i == 4))

        # ---- projection units (thunks, one PSUM tile each) ----
        KTW = (512, 512, 512, 512, 128)
        KTO = (0, 512, 1024, 1536, 2048)

        def qt_unit(jb, nb):
            ps = pj.tile([C, 512], DT_F32, name="pj")
            for kb in range(4):
                nc.tensor.matmul(
                    ps[:],
                    lhsT=wq[:, 512 * jb + C * kb:512 * jb + C * (kb + 1)],
                    rhs=xtd[:, 2048 * nb + 512 * kb:2048 * nb + 512 * (kb + 1)],
                    start=(kb == 0), stop=(kb == 3))
            cp(qt[:, ROWS * jb + 512 * nb:ROWS * jb + 512 * (nb + 1)], ps[:])

        def kt_unit(jb, nb):
            off, w = KTO[nb], KTW[nb]
            ps = pj.tile([C, 512], DT_F32, name="pj")
            for kb in range(4):
                nc.tensor.matmul(
                    ps[:, :w],
                    lhsT=wk[:, 512 * jb + C * kb:512 * jb + C * (kb + 1)],
                    rhs=xte[:, 4 * off + w * kb:4 * off + w * (kb + 1)],
                    start=(kb == 0), stop=(kb == 3))
            cp(kt[:, EROWS * jb + off:EROWS * jb + off + w], ps[:, :w])

        def v_unit(hp, r):
            c = min(r // 4, 4)
            ps = pj.tile([C, 512], DT_F32, name="pj")
            for kb in range(4):
                xb = 4 * KTO[c] + KTW[c] * kb + (C * r - KTO[c])
                nc.tensor.matmul(
                    ps[:, :C],
                    lhsT=xte[:, xb:xb + C],
                    rhs=wv[:, 512 * hp + C * kb:512 * hp + C * (kb + 1)],
                    start=(kb == 0), stop=(kb == 3))
            dst = v4[hp][:, 2 * VST * r:2 * VST * r + 2 * VST]
            cp(dst.rearrange("p (e o) -> p e o", e=2)[:, :, 0:DH],
               ps[:, :C].rearrange("p (e d) -> p e d", e=2), kind="small")

        def slab_unit_list(jb):
            # need-ordered: group n needs qt(n), kt(n) by jk=4n; v(r) by jk=r
            units = []
            for n in range(4):
                units.append(lambda n=n: qt_unit(jb, n))
                units.append(lambda n=n: kt_unit(jb, n))
                for r in range(4 * n, 4 * n + 4):
                    units.append(lambda r=r: v_unit(jb, r))
            units.append(lambda: kt_unit(jb, 4))
            units.append(lambda: v_unit(jb, 16))
            return units

        LOOK = 3  # pump lookahead (units) to hide copy->consumer latency

        # global just-in-time unit stream: phases pump across slab boundaries
        all_units = []
        for jb in range(4):
            all_units.extend(slab_unit_list(jb))
        cursor = [0]

        def pump_to(target):
            target = min(target, len(all_units))
            while cursor[0] < target:
                all_units[cursor[0]]()
                cursor[0] += 1

        # ---- attention per head pair, pumping its own slab just-in-time ----
        # PV computes out directly in [query, head-dim] layout: the exp tile
        # pb (keys on partitions) is the STATIONARY operand, v the moving one.
        # out[q, d] lands on query partitions; col 64 is the softmax
        # denominator (ones column of v), so normalization is a per-chunk
        # reciprocal + tensor_scalar_mul -- no transposes.
        def attention(hp):
            def pump(jk):
                # everything group jk//4 needs, plus lookahead; the linear
                # floor keeps the unit stream continuous across phases
                need = 26 * hp + 6 * (jk // 4) + (jk % 4) + 3
                b = 17 * hp + jk
                lin = (104 * (b + 1) + 67) // 68
                look = LOOK if (hp < 3 or jk < 10) else 0
                pump_to(max(need, lin) + look)

            pv_tiles = {}

            def norm_chunk(jq):
                # both heads of chunk jq are complete: normalize into outg
                pvt = pv_tiles.pop(jq)
                g, c = jq // 4, jq % 4
                rc = rcp.tile([C, 2], DT_F32, name="rc")
                pv3 = pvt.rearrange("p (e o) -> p e o", o=VST)
                nc.vector.reciprocal(
                    rc.rearrange("p (e o) -> p e o", o=1), pv3[:, :, DH:DH + 1])
                for e in range(2):
                    h = 2 * hp + e
                    nc.vector.tensor_scalar_mul(
                        outg[g][:, 512 * c + DH * h:512 * c + DH * (h + 1)],
                        pvt[:, VST * e:VST * e + DH], rc[:, e:e + 1])
                if hp == 3:
                    d2 = out_d[512 * g + C * c:512 * g + C * (c + 1), :]
                    nc.sync.dma_start(
                        d2, outg[g][:, 512 * c:512 * (c + 1)])

            def emit_pv(jq, pb_lo, pb_hi):
                # all 4 contributions of chunk jq, sequentially per head so
                # only one PSUM accumulation group is ever pending per bank
                pvt = pvp.tile([C, 2 * VST], DT_F32, name="pv")
                pv_tiles[jq] = pvt
                lo_off = 0 if jq == 0 else C
                for e in range(2):
                    nc.tensor.matmul(
                        pvt[:, VST * e:VST * (e + 1)],
                        lhsT=pb_lo[:, 256 * e + lo_off:256 * e + lo_off + C],
                        rhs=v4[hp][:, 2 * VST * jq + VST * e:2 * VST * jq + VST * (e + 1)],
                        start=True, stop=False)
                    nc.tensor.matmul(
                        pvt[:, VST * e:VST * (e + 1)],
                        lhsT=pb_hi[:, 256 * e:256 * e + C],
                        rhs=v4[hp][:, 2 * VST * (jq + 1) + VST * e:2 * VST * (jq + 1) + VST * (e + 1)],
                        start=False, stop=True)
                norm_chunk(jq)

            pbs = {}
            for jk in range(NKC):
                pump(jk)
                ncols = C if jk in (0, NKC - 1) else 2 * C
                qcol0 = ROWS * hp + max(0, (jk - 1) * C)
                # two PSUM banks per score tile: each head's matmul drains
                # from a different PE row-offset, and the hardware requires
                # distinct banks for distinct row-offsets
                sc = scp.tile([C, 1024], DT_F32, name="sc")
                for e in range(2):
                    po = DH * e
                    nc.tensor.matmul(
                        sc[:, 512 * e:512 * e + ncols],
                        lhsT=kt[po:po + DH, EROWS * hp + C * jk:EROWS * hp + C * (jk + 1)],
                        rhs=qt[po:po + DH, qcol0:qcol0 + ncols],
                        start=True, stop=True)
                pb = pbp.tile([C, 512], DT_BF, name="pb")
                sc3 = sc.rearrange("p (e q) -> p e q", e=2)
                pb3 = pb.rearrange("p (e q) -> p e q", e=2)
                nc.scalar.activation(pb3[:, :, 0:ncols], sc3[:, :, 0:ncols], EXP)
                msk = m0_sb if jk == 0 else trm_sb
                nc.gpsimd.tensor_mul(
                    pb3[:, :, 0:C], pb3[:, :, 0:C],
                    msk.rearrange("p (e q) -> p e q", e=2))
                pbs[jk] = pb
                if jk >= 2:
                    emit_pv(jk - 2, pbs.pop(jk - 2), pbs[jk - 1])
            emit_pv(NCH - 1, pbs.pop(NCH - 1), pbs[NCH])

        # ---- schedule ----
        for hp in range(4):
            attention(hp)

    nc.finalize()
    return nc


def _get_program():
    if "nc" not in _CACHED:
        _CACHED["nc"] = _build_program()
    return _CACHED["nc"]


def _chunkify(a, widths):
    # [512, N] -> [128, 4*N] chunk-major: chunk c (width w) occupies cols
    # [4*off_c, 4*off_c + 4*w) laid out kb-major inside.
    blocks = []
    o = 0
    for w in widths:
        blk = a[:, o:o + w].reshape(4, C, w).transpose(1, 0, 2).reshape(C, 4 * w)
        blocks.append(blk)
        o += w
    return np.ascontiguousarray(np.concatenate(blocks, axis=1))


def _host_prep(decoder_states, hidden_states, Wq, Wk, Wv):
    wqt = _chunkify(np.ascontiguousarray(Wq.T).astype(BF16), [C] * 4)
    wkt = _chunkify(np.ascontiguousarray(
        (Wk / np.sqrt(np.float32(DH))).T).astype(BF16), [C] * 4)
    wvt = _chunkify(np.ascontiguousarray(Wv.T).astype(BF16), [C] * 4)
    k = np.arange(C, dtype=np.int32)
    tri = (k[None, :] >= k[:, None]).astype(BF16)   # tri[key, query]
    trm = np.concatenate([tri, tri], axis=1)
    ones = np.ones((C, 2 * C), dtype=BF16)
    zeros = np.zeros((C, 2 * C), dtype=BF16)
    iden = np.eye(C, dtype=BF16)

    in_maps = []
    for core in range(8):
        b, q = core // 4, core % 4
        r0 = q * ROWS
        xt_dec = _chunkify(np.ascontiguousarray(
            decoder_states[b, r0:r0 + ROWS, :].T).astype(BF16), [512] * 4)
        if q == 0:
            slab = np.concatenate(
                [np.zeros((C, H), np.float32), hidden_states[b, 0:ROWS, :]], axis=0)
        else:
            slab = hidden_states[b, r0 - C:r0 + ROWS, :]
        xt_enc = _chunkify(np.ascontiguousarray(slab.T).astype(BF16),
                           [512, 512, 512, 512, C])
        in_maps.append({
            "xt_dec": xt_dec, "xt_enc": xt_enc,
            "wqt": wqt, "wkt": wkt, "wvt": wvt,
            "trm": trm, "m0": zeros if q == 0 else ones, "iden": iden,
        })
    return in_maps


def kernel(decoder_states, hidden_states, attention_mask, Wq, Wk, Wv,
           _trace=False, _trace_kwargs=None):
    nc = _get_program()
    in_maps = _host_prep(decoder_states, hidden_states, Wq, Wk, Wv)
    res = run_bass_kernel_spmd(nc, in_maps, core_ids=list(range(8)),
                               trace=_trace, **(_trace_kwargs or {}))
    out = np.empty((B, T, H), dtype=np.float32)
    for core in range(8):
        b, q = core // 4, core % 4
        out[b, q * ROWS:(q + 1) * ROWS, :] = res.results[core]["out"].astype(np.float32)
    if _trace:
        _CACHED["last_results"] = res
    return out



# revision 10
# speedup vs baseline: 1.0520x; 1.0520x over previous
"""Trainium2 Bass kernel v3 for nn_LocalCrossAttention (chunked local cross-attn).

Problem (hardcoded): B=2, T=S=8192, HIDDEN=512, NUM_HEADS=8, HEAD_SIZE=64,
CHUNK=128, N_BEFORE=1, N_AFTER=0, attention_mask == ones.

Sharding: 8 cores = batch(2) x sequence-quarters(4). Each core: 2048 query
rows (16 chunks) of one batch element, all 8 heads, 1-chunk halo of encoder
rows. Quarter 0's halo is structurally fully masked -> zeros + zero mask.

Design (CoreSim-driven; ~81us/core in the cost-model sim vs ~145us for the
original staged version; HW-verified, L2 ~5.6e-3):
  - DIRECT-LAYOUT PV: the exp tile pb (keys on partitions) is the STATIONARY
    matmul operand and V the moving one, so out = pb.T @ V lands directly as
    [query, head-dim] in PSUM. No transposes, no staging copies. V carries a
    ones column, so PSUM col 64 is the softmax denominator: normalization is
    one batched reciprocal + tensor_scalar_mul per chunk.
  - per key chunk: ONE 2-bank PSUM score tile (the HW cannot drain two
    different PE row-offsets into one bank - the two heads sit at row
    offsets 0/64), ONE strided exp (ACT), ONE strided mask-mul (GPSIMD/Pool,
    which is otherwise idle and may not touch PSUM).
  - per-chunk PV accumulation groups run strictly sequentially per bank
    (start->stop per head) - concurrent pending groups in one bank are
    illegal.
  - projections stream as 26 "units" per head-pair phase, pumped just-in-time
    (need-based + linear floor + lookahead) between attention bodies so the
    PE never starves while ACT runs exp; phases pump across boundaries.
  - chunk-major contiguous SBUF/DRAM layouts for every input DMA: interval-
    hull dependency tracking otherwise fabricates WAR chains that head-of-
    line-block the in-order DMA queue.
  - input DMAs ordered to match phase-0 consumption (first matmul at ~3us);
    PE warmup matmuls during the DMA lead-in; per-chunk output DMA (bf16,
    host casts to fp32) overlapped with compute; final group drained
    per-chunk to shorten the epilogue.
"""

import os
import sys

import numpy as np
import ml_dtypes

for _p in ("/opt/trn_rl_repo",):
    if _p not in sys.path and os.path.isdir(_p):
        sys.path.append(_p)

import concourse.bass as bass
import concourse.bacc as bacc
import concourse.mybir as mybir
from concourse.tile import TileContext
from concourse.bass_utils import run_bass_kernel_spmd

from contextlib import ExitStack

BF16 = ml_dtypes.bfloat16
DT_BF = mybir.dt.bfloat16
DT_F32 = mybir.dt.float32
EXP = mybir.ActivationFunctionType.Exp

B, T, H = 2, 8192, 512
NH, DH, C = 8, 64, 128
ROWS = 2048          # query rows per core
NCH = ROWS // C      # 16 local query chunks
EROWS = ROWS + C     # encoder rows per core incl. halo
NKC = EROWS // C     # 17 key chunks
VST = DH + 2         # v stride per head (64 vals + ones col + pad for alignment)

# One DRAM input tensor, laid out in phase-0 consumption order so a handful
# of need-ordered DMA slices feed the PE just in time (each dma_start costs
# ~650ns of serialized HWDGE issue, so fewer+bigger is cheaper, but the lead
# slices stay small to cut first-matmul latency).
# Segments: (logical tensor, start col, width) in stream order.
MEGA_SEGS = [
    ("wq", 0, 512), ("xtd", 0, 1024),
    ("xtd", 1024, 1024),
    ("wk", 0, 512), ("xte", 0, 1024),
    ("xte", 1024, 1024),
    ("wv", 0, 512), ("trm", 0, 256), ("m0", 0, 256),
    ("xtd", 2048, 2048), ("xte", 2048, 2048),
    ("xtd", 4096, 2048), ("xte", 4096, 2048),
    ("xtd", 6144, 2048), ("xte", 6144, 2048),
    ("xte", 8192, 512), ("wq", 512, 512), ("wk", 512, 512), ("wv", 512, 512),
    ("wq", 1024, 512), ("wk", 1024, 512), ("wv", 1024, 512),
    ("wq", 1536, 512), ("wk", 1536, 512), ("wv", 1536, 512),
]
# DMA slice boundaries (segment index ranges)
MEGA_SLICES = [(0, 2), (2, 3), (3, 5), (5, 6), (6, 9), (9, 10), (10, 11),
               (11, 12), (12, 13), (13, 14), (14, 15), (15, 19), (19, 22),
               (22, 25)]
MEGA_COLS = sum(w for _, _, w in MEGA_SEGS)

# per-tensor segment lookup: name -> list of (start, width, mega_off)
_SEGMAP = {}
_off = 0
for _name, _s0, _w in MEGA_SEGS:
    _SEGMAP.setdefault(_name, []).append((_s0, _w, _off))
    _off += _w

_CACHED = {}


def _mega_off(name, c0, w):
    """mega column offset for logical tensor `name` cols [c0, c0+w)."""
    for s0, sw, off in _SEGMAP[name]:
        if s0 <= c0 and c0 + w <= s0 + sw:
            return off + (c0 - s0)
    raise AssertionError(f"{name}[{c0}:{c0 + w}] crosses mega segments")


def _build_program():
    nc = bacc.Bacc("TRN2", target_bir_lowering=False, debug=False,
                   enable_asserts=False, num_devices=8)

    mega_d = nc.dram_tensor("mega", [C, MEGA_COLS], DT_BF, kind="ExternalInput").ap()
    out_d = nc.dram_tensor("out", [ROWS, H], DT_BF, kind="ExternalOutput").ap()
    # [p, chunk, h] view for pair-granular output DMAs
    out_dp = out_d.rearrange("(a p) h -> p a h", p=C)

    with TileContext(nc) as tc, ExitStack() as ctx:
        const = ctx.enter_context(tc.tile_pool(name="const", bufs=1))

        # ---- resident SBUF tensors ----
        mega = const.tile([C, MEGA_COLS], DT_BF, name="mega")
        qt = const.tile([C, 4 * ROWS], DT_BF, name="qt")
        kt = const.tile([C, 4 * EROWS], DT_BF, name="kt")
        v4 = [const.tile([C, NKC * 2 * VST], DT_BF, name=f"v{hp}") for hp in range(4)]
        outg = [const.tile([C, 4 * H], DT_BF, name=f"og{g}") for g in range(4)]

        def mv(name, c0, w):
            o = _mega_off(name, c0, w)
            return mega[:, o:o + w]

        # ones columns of v tiles (d==64 of each head slot; 65 is pad)
        for hp in range(4):
            vv = v4[hp].rearrange("p (n o) -> p n o", o=VST)
            nc.vector.memset(vv[:, :, DH:DH + 2], 1.0)

        # ---- input DMAs: consumption-ordered slices of the mega stream ----
        # Each slice writes one contiguous interval: interval-hull dependency
        # tracking never creates false WAR chains between later input DMAs
        # and earlier reads.
        seg_cols = []
        o = 0
        for _, _, w in MEGA_SEGS:
            seg_cols.append(o)
            o += w
        for (sa, sb_) in MEGA_SLICES:
            a = seg_cols[sa]
            b = seg_cols[sb_ - 1] + MEGA_SEGS[sb_ - 1][2]
            nc.sync.dma_start(mega[:, a:b], mega_d[:, a:b])

        # ---- engine-rotating PSUM->SBUF copy helper (weighted by size) ----
        ROT_BIG = [nc.scalar, nc.vector]
        ROT_SMALL = [nc.vector]
        ROT_TR = [nc.vector, nc.scalar]

        def cp(dst, src, kind="big", rots={"big": [0], "small": [0], "tr": [0]}):
            lst = {"big": ROT_BIG, "small": ROT_SMALL, "tr": ROT_TR}[kind]
            r = rots[kind]
            e = lst[r[0] % len(lst)]
            r[0] += 1
            if e is nc.scalar:
                e.copy(dst, src)
            else:
                e.tensor_copy(dst, src)

        pj = ctx.enter_context(tc.tile_pool(name="pj", bufs=2, space="PSUM"))
        scp = ctx.enter_context(tc.tile_pool(name="scp", bufs=2, space="PSUM"))
        pvp = ctx.enter_context(tc.tile_pool(name="pvp", bufs=2, space="PSUM"))
        pbp = ctx.enter_context(tc.tile_pool(name="pbp", bufs=4))
        rcp = ctx.enter_context(tc.tile_pool(name="rcp", bufs=4))

        # ---- PE warmup: dummy matmuls while input DMAs stream ----
        warm = const.tile([C, 512], DT_BF, name="warm")
        nc.vector.memset(warm[:], 0.0)
        wps = pj.tile([C, 512], DT_F32, name="pj")
        for i in range(5):
            nc.tensor.matmul(wps[:], lhsT=warm[:, 0:C], rhs=warm[:],
                             start=(i == 0), stop=(i == 4))

        # ---- projection units (thunks, one PSUM tile each) ----
        KTW = (512, 512, 512, 512, 128)
        KTO = (0, 512, 1024, 1536, 2048)

        def qt_unit(jb, nb):
            ps = pj.tile([C, 512], DT_F32, name="pj")
            for kb in range(4):
                nc.tensor.matmul(
                    ps[:],
                    lhsT=mv("wq", 512 * jb + C * kb, C),
                    rhs=mv("xtd", 2048 * nb + 512 * kb, 512),
                    start=(kb == 0), stop=(kb == 3))
            cp(qt[:, ROWS * jb + 512 * nb:ROWS * jb + 512 * (nb + 1)], ps[:])

        def kt_unit(jb, nb):
            off, w = KTO[nb], KTW[nb]
            ps = pj.tile([C, 512], DT_F32, name="pj")
            for kb in range(4):
                nc.tensor.matmul(
                    ps[:, :w],
                    lhsT=mv("wk", 512 * jb + C * kb, C),
                    rhs=mv("xte", 4 * off + w * kb, w),
                    start=(kb == 0), stop=(kb == 3))
            cp(kt[:, EROWS * jb + off:EROWS * jb + off + w], ps[:, :w])

        def v_unit(hp, r):
            c = min(r // 4, 4)
            ps = pj.tile([C, 512], DT_F32, name="pj")
            for kb in range(4):
                xb = 4 * KTO[c] + KTW[c] * kb + (C * r - KTO[c])
                nc.tensor.matmul(
                    ps[:, :C],
                    lhsT=mv("xte", xb, C),
                    rhs=mv("wv", 512 * hp + C * kb, C),
                    start=(kb == 0), stop=(kb == 3))
            dst = v4[hp][:, 2 * VST * r:2 * VST * r + 2 * VST]
            cp(dst.rearrange("p (e o) -> p e o", e=2)[:, :, 0:DH],
               ps[:, :C].rearrange("p (e d) -> p e d", e=2), kind="small")

        def slab_unit_list(jb):
            # need-ordered: group n needs qt(n), kt(n) by jk=4n; v(r) by jk=r
            units = []
            for n in range(4):
                units.append(lambda n=n: qt_unit(jb, n))
                units.append(lambda n=n: kt_unit(jb, n))
                for r in range(4 * n, 4 * n + 4):
                    units.append(lambda r=r: v_unit(jb, r))
            units.append(lambda: kt_unit(jb, 4))
            units.append(lambda: v_unit(jb, 16))
            return units

        LOOK = 3  # pump lookahead (units) to hide copy->consumer latency

        # global just-in-time unit stream: phases pump across slab boundaries
        all_units = []
        for jb in range(4):
            all_units.extend(slab_unit_list(jb))
        cursor = [0]

        def pump_to(target):
            target = min(target, len(all_units))
            while cursor[0] < target:
                all_units[cursor[0]]()
                cursor[0] += 1

        # ---- attention per head pair, pumping its own slab just-in-time ----
        # PV computes out directly in [query, head-dim] layout: the exp tile
        # pb (keys on partitions) is the STATIONARY operand, v the moving one.
        # out[q, d] lands on query partitions; col 64 is the softmax
        # denominator (ones column of v), so normalization is a per-chunk
        # reciprocal + tensor_scalar_mul -- no transposes.
        def attention(hp):
            def pump(jk):
                # everything group jk//4 needs, plus lookahead; the linear
                # floor keeps the unit stream continuous across phases
                need = 26 * hp + 6 * (jk // 4) + (jk % 4) + 3
                b = 17 * hp + jk
                lin = (104 * (b + 1) + 67) // 68
                look = LOOK if (hp < 3 or jk < 10) else 0
                pump_to(max(need, lin) + look)

            def norm_chunk(pvt, base, jq):
                # normalize a single chunk (cols base..base+2*VST of pvt)
                g, c = jq // 4, jq % 4
                rc = rcp.tile([C, 2], DT_F32, name="rc")
                pv3 = pvt[:, base:base + 2 * VST].rearrange(
                    "p (e o) -> p e o", o=VST)
                rc3 = rc.rearrange("p (e o) -> p e o", o=1)
                nc.vector.reciprocal(rc3, pv3[:, :, DH:DH + 1])
                dst = outg[g].rearrange(
                    "p (ch h o) -> p ch h o", ch=4, o=DH)[:, c, 2 * hp:2 * hp + 2, :]
                b_in, b_rc = bass.broadcast_tensor_aps(pv3[:, :, 0:DH], rc3)
                nc.vector.tensor_mul(dst, b_in, b_rc)
                if hp == 3:
                    d2 = out_d[512 * g + C * c:512 * g + C * (c + 1), :]
                    nc.sync.dma_start(
                        d2, outg[g][:, 512 * c:512 * (c + 1)])

            def norm_pair(pvt, jq):
                # normalize chunks jq, jq+1 (pair tile): one reciprocal,
                # one broadcast multiply, one pair output DMA
                g, c = jq // 4, jq % 4     # c in {0, 2}
                rc = rcp.tile([C, 4], DT_F32, name="rc")
                pv4 = pvt[:, 0:4 * VST].rearrange(
                    "p (i e o) -> p i e o", i=2, o=VST)
                rc4 = rc.rearrange("p (i e o) -> p i e o", i=2, o=1)
                nc.vector.reciprocal(rc4, pv4[:, :, :, DH:DH + 1])
                dst = outg[g].rearrange(
                    "p (ch h o) -> p ch h o", ch=4, o=DH)[:, c:c + 2, 2 * hp:2 * hp + 2, :]
                b_in, b_rc = bass.broadcast_tensor_aps(pv4[:, :, :, 0:DH], rc4)
                nc.vector.tensor_mul(dst, b_in, b_rc)
                if hp == 3:
                    d2 = out_dp[:, 4 * g + c:4 * g + c + 2, :]
                    nc.sync.dma_start(
                        d2, outg[g][:, 512 * c:512 * (c + 2)].rearrange(
                            "p (i o) -> p i o", i=2))

            def emit_pv_pair(jq):
                # all 8 contributions of chunks jq, jq+1 into one pair tile,
                # sequentially per head so only one PSUM accumulation group
                # is ever pending per bank
                pvt = pvp.tile([C, 512], DT_F32, name="pv")
                # no per-chunk split: reading a PSUM bank while it still has
                # a pending accumulation group is an HW-only hazard
                split = False
                for i in range(2):
                    jc = jq + i
                    pb_lo, pb_hi = pbs[jc], pbs[jc + 1]
                    lo_off = 0 if jc == 0 else C
                    for e in range(2):
                        o = 2 * VST * i + VST * e
                        nc.tensor.matmul(
                            pvt[:, o:o + VST],
                            lhsT=pb_lo[:, 256 * e + lo_off:256 * e + lo_off + C],
                            rhs=v4[hp][:, 2 * VST * jc + VST * e:2 * VST * jc + VST * (e + 1)],
                            start=True, stop=False)
                        nc.tensor.matmul(
                            pvt[:, o:o + VST],
                            lhsT=pb_hi[:, 256 * e:256 * e + C],
                            rhs=v4[hp][:, 2 * VST * (jc + 1) + VST * e:2 * VST * (jc + 1) + VST * (e + 1)],
                            start=False, stop=True)
                    if split:
                        # shorten the epilogue: norm+DMA each chunk of the
                        # final pair as soon as its PV closes
                        norm_chunk(pvt, 2 * VST * i, jc)
                pbs.pop(jq)
                pbs.pop(jq + 1)
                if not split:
                    norm_pair(pvt, jq)

            pbs = {}
            for jk in range(NKC):
                pump(jk)
                ncols = C if jk in (0, NKC - 1) else 2 * C
                qcol0 = ROWS * hp + max(0, (jk - 1) * C)
                # two PSUM banks per score tile: each head's matmul drains
                # from a different PE row-offset, and the hardware requires
                # distinct banks for distinct row-offsets
                sc = scp.tile([C, 1024], DT_F32, name="sc")
                for e in range(2):
                    po = DH * e
                    nc.tensor.matmul(
                        sc[:, 512 * e:512 * e + ncols],
                        lhsT=kt[po:po + DH, EROWS * hp + C * jk:EROWS * hp + C * (jk + 1)],
                        rhs=qt[po:po + DH, qcol0:qcol0 + ncols],
                        start=True, stop=True)
                pb = pbp.tile([C, 512], DT_BF, name="pb")
                sc3 = sc.rearrange("p (e q) -> p e q", e=2)
                pb3 = pb.rearrange("p (e q) -> p e q", e=2)
                nc.scalar.activation(pb3[:, :, 0:ncols], sc3[:, :, 0:ncols], EXP)
                msk = mv("m0", 0, 2 * C) if jk == 0 else mv("trm", 0, 2 * C)
                nc.gpsimd.tensor_mul(
                    pb3[:, :, 0:C], pb3[:, :, 0:C],
                    msk.rearrange("p (e q) -> p e q", e=2))
                pbs[jk] = pb
                # emit at odd jk: the pair needs pb[jk-1], which had a full
                # iteration (score+exp+mask of jk) to drain -- the PE never
                # waits on the current exp
                if jk >= 3 and jk % 2 == 1:
                    emit_pv_pair(jk - 3)
            emit_pv_pair(NCH - 2)

        # ---- schedule ----
        for hp in range(4):
            attention(hp)

    nc.finalize()
    return nc


def _get_program():
    if "nc" not in _CACHED:
        _CACHED["nc"] = _build_program()
    return _CACHED["nc"]


def _chunkify(a, widths):
    # [512, N] -> [128, 4*N] chunk-major: chunk c (width w) occupies cols
    # [4*off_c, 4*off_c + 4*w) laid out kb-major inside.
    blocks = []
    o = 0
    for w in widths:
        blk = a[:, o:o + w].reshape(4, C, w).transpose(1, 0, 2).reshape(C, 4 * w)
        blocks.append(blk)
        o += w
    return np.ascontiguousarray(np.concatenate(blocks, axis=1))


def _host_prep(decoder_states, hidden_states, Wq, Wk, Wv):
    wqt = _chunkify(np.ascontiguousarray(Wq.T).astype(BF16), [C] * 4)
    wkt = _chunkify(np.ascontiguousarray(
        (Wk / np.sqrt(np.float32(DH))).T).astype(BF16), [C] * 4)
    wvt = _chunkify(np.ascontiguousarray(Wv.T).astype(BF16), [C] * 4)
    k = np.arange(C, dtype=np.int32)
    tri = (k[None, :] >= k[:, None]).astype(BF16)   # tri[key, query]
    trm = np.concatenate([tri, tri], axis=1)
    ones = np.ones((C, 2 * C), dtype=BF16)
    zeros = np.zeros((C, 2 * C), dtype=BF16)

    in_maps = []
    for core in range(8):
        b, q = core // 4, core % 4
        r0 = q * ROWS
        xt_dec = _chunkify(np.ascontiguousarray(
            decoder_states[b, r0:r0 + ROWS, :].T).astype(BF16), [512] * 4)
        if q == 0:
            slab = np.concatenate(
                [np.zeros((C, H), np.float32), hidden_states[b, 0:ROWS, :]], axis=0)
        else:
            slab = hidden_states[b, r0 - C:r0 + ROWS, :]
        xt_enc = _chunkify(np.ascontiguousarray(slab.T).astype(BF16),
                           [512, 512, 512, 512, C])
        logical = {"wq": wqt, "wk": wkt, "wv": wvt, "xtd": xt_dec,
                   "xte": xt_enc, "trm": trm, "m0": zeros if q == 0 else ones}
        mega = np.concatenate(
            [logical[name][:, s0:s0 + w] for name, s0, w in MEGA_SEGS], axis=1)
        in_maps.append({"mega": np.ascontiguousarray(mega)})
    return in_maps


def kernel(decoder_states, hidden_states, attention_mask, Wq, Wk, Wv,
           _trace=False, _trace_kwargs=None):
    nc = _get_program()
    in_maps = _host_prep(decoder_states, hidden_states, Wq, Wk, Wv)
    res = run_bass_kernel_spmd(nc, in_maps, core_ids=list(range(8)),
                               trace=_trace, **(_trace_kwargs or {}))
    out = np.empty((B, T, H), dtype=np.float32)
    for core in range(8):
        b, q = core // 4, core % 4
        out[b, q * ROWS:(q + 1) * ROWS, :] = res.results[core]["out"].astype(np.float32)
    if _trace:
        _CACHED["last_results"] = res
    return out

